# revision 1
# baseline (speedup 1.0000x reference)
"""Trainium2 Bass kernel for nn_BucketedGoWatti (sparse windowed attention pooling).

Math (B=4, L=4096, T=32, DH=1024, DG=256, DP=256, WIN=1024, STRIDE=256, W=13):
  q  = G @ Wq_core;  k = H @ Wk_core (window-independent)
  logits[b,w,t,l] = slice of global  s * (q @ Wk_core^T) @ H^T
  alpha = softmax in window; Zw[b,t,w,:] = alpha @ Hw
  wlog[b,t,w] = Zw . qw2,  qw2 = (G@Wq_win) @ Wk_win^T * DH^-0.5
  Z = softmax_w(wlog) @ Zw   (tiny; done on host at gather time)

Sharding: core c -> batch b=c//2, window half c%2 (even: windows 0-6 over
l in [0,2560); odd: windows 6-12 over l in [1536,4096); window 6 duplicated
so all 8 cores run one SPMD program shape). Cross-window combine on host.

Precision: big matmuls in float32r (~1.5e-4 rel); qw2 path bf16 (negligible
through the 13-way combine softmax); softmax/normalization fp32.
"""
import numpy as np
import ml_dtypes
from contextlib import ExitStack

import concourse.bacc as bacc
import concourse.tile as tile
import concourse.mybir as mybir
import concourse.masks as masks
from concourse.bass_utils import run_bass_kernel_spmd

F32 = mybir.dt.float32
F32R = mybir.dt.float32r
BF16 = mybir.dt.bfloat16
ActFn = mybir.ActivationFunctionType
Alu = mybir.AluOpType

B, L, T = 4, 4096, 32
DH, DG, DP = 1024, 256, 256
WIN, STRIDE = 1024, 256
W = (L - WIN) // STRIDE + 1          # 13
SPAN = 2560                          # per-core l-span
NLT = SPAN // 128                    # 20 l-tiles
NCH = SPAN // 256                    # 10 logits chunks of 256
WLOC = 7                             # windows per core
NDT = DH // 128                      # 8 d-tiles
S_CORE = 1.0 / float(np.sqrt(DP))
S_WIN = 1.0 / float(np.sqrt(DH))

_CACHE = {}


def _build(with_mask: bool, stage: int = 99):
    nc = bacc.Bacc("TRN2", debug=False, target_bir_lowering=False)

    Hn_d = nc.dram_tensor("Hn", [SPAN, DH], F32R, kind="ExternalInput")
    HT_d = nc.dram_tensor("HT", [DH, SPAN], F32R, kind="ExternalInput")
    GT_d = nc.dram_tensor("GT", [DG, T], F32R, kind="ExternalInput")
    Wqc_d = nc.dram_tensor("Wqc", [DG, DP], F32R, kind="ExternalInput")
    WkcT_d = nc.dram_tensor("WkcT", [DP, DH], F32R, kind="ExternalInput")
    Wqw_d = nc.dram_tensor("Wqw", [DG, DH], F32R, kind="ExternalInput")
    WkwT_d = nc.dram_tensor("WkwT", [DH, DH], BF16, kind="ExternalInput")
    if with_mask:
        mb_d = nc.dram_tensor("maskbias", [1, SPAN], F32R, kind="ExternalInput")
        ones_d = nc.dram_tensor("onesrow", [1, T], F32R, kind="ExternalInput")
    zw_d = nc.dram_tensor("Zw_out", [WLOC * T, DH], F32, kind="ExternalOutput")
    wl_d = nc.dram_tensor("wlog_out", [T, WLOC], F32, kind="ExternalOutput")

    with tile.TileContext(nc) as tc, ExitStack() as ctx:
        const = ctx.enter_context(tc.tile_pool(name="const", bufs=1))
        hpool = ctx.enter_context(tc.tile_pool(name="hpool", bufs=16))
        htp = ctx.enter_context(tc.tile_pool(name="htp", bufs=12))
        sb = ctx.enter_context(tc.tile_pool(name="sb", bufs=1))
        sexp = ctx.enter_context(tc.tile_pool(name="sexp", bufs=1))
        pj = ctx.enter_context(tc.tile_pool(name="pj", bufs=2, space="PSUM"))
        lg = ctx.enter_context(tc.tile_pool(name="lg", bufs=2, space="PSUM"))
        zp = ctx.enter_context(tc.tile_pool(name="zp", bufs=4, space="PSUM"))

        # ---- small resident inputs ----
        ident = const.tile([128, 128], F32, tag="ident")
        masks.make_identity(nc, ident[:])
        gt = const.tile([128, 2 * T], F32R, tag="gt")
        wqc = const.tile([128, 2 * DP], F32R, tag="wqc")
        wkcT = const.tile([128, 2 * DH], F32R, tag="wkcT")
        wqw = const.tile([128, 2 * DH], F32R, tag="wqw")
        wkwT = const.tile([128, NDT * DH], BF16, tag="wkwT")
        for g in range(2):
            nc.gpsimd.dma_start(gt[:, g * T:(g + 1) * T], GT_d.ap()[g * 128:(g + 1) * 128, :])
            nc.gpsimd.dma_start(wqc[:, g * DP:(g + 1) * DP], Wqc_d.ap()[g * 128:(g + 1) * 128, :])
            nc.gpsimd.dma_start(wkcT[:, g * DH:(g + 1) * DH], WkcT_d.ap()[g * 128:(g + 1) * 128, :])
            nc.gpsimd.dma_start(wqw[:, g * DH:(g + 1) * DH], Wqw_d.ap()[g * 128:(g + 1) * 128, :])

        if with_mask:
            mbias = const.tile([1, SPAN], F32R, tag="mbias")
            onesr = const.tile([1, T], F32R, tag="onesr")
            nc.gpsimd.dma_start(mbias[:], mb_d.ap())
            nc.gpsimd.dma_start(onesr[:], ones_d.ap())

        # ---- q^T then qk^T ----
        qT = []
        for p in range(2):
            ps_ = pj.tile([128, 512], F32, tag="pj")
            for g in range(2):
                nc.tensor.matmul(ps_[:, :T], wqc[:, g * DP + p * 128:g * DP + (p + 1) * 128],
                                 gt[:, g * T:(g + 1) * T], start=(g == 0), stop=(g == 1))
            t_ = sb.tile([128, T], F32R, tag=f"qT{p}")
            nc.scalar.activation(t_[:], ps_[:, :T], ActFn.Identity, scale=S_CORE)
            qT.append(t_)
        qkT = []
        for i in range(NDT):
            ps_ = pj.tile([128, 512], F32, tag="pj")
            for p in range(2):
                nc.tensor.matmul(ps_[:, :T], wkcT[:, p * DH + i * 128:p * DH + (i + 1) * 128],
                                 qT[p][:], start=(p == 0), stop=(p == 1))
            t_ = sb.tile([128, T], F32R, tag=f"qkT{i}")
            nc.vector.tensor_copy(t_[:], ps_[:, :T])
            qkT.append(t_)
        if stage == 1:
            dbg = sb.tile([128, NDT * T], F32, tag="dbg")
            for i in range(NDT):
                nc.vector.tensor_copy(dbg[:, i * T:(i + 1) * T], qkT[i][:].bitcast(F32))
            nc.sync.dma_start(zw_d.ap()[:128, :NDT * T], dbg[:])

        # ---- logits chunks + exp (+ per-chunk sums) ----
        hn = []
        if stage >= 2:
            expLs, csums = [], []
            for c in range(NCH):
                ec_ = sexp.tile([T, 256], F32, tag=f"expL{c}")
                cs_ = sexp.tile([T, 1], F32, tag=f"csum{c}")
                expLs.append(ec_)
                csums.append(cs_)
            ht = {}
            for cc in range(SPAN // 512):
                for i in range(NDT):
                    t_ = htp.tile([128, 512], F32R, tag="ht")
                    nc.sync.dma_start(t_[:], HT_d.ap()[i * 128:(i + 1) * 128,
                                                       cc * 512:(cc + 1) * 512])
                    ht[(cc, i)] = t_
                if cc == 0 and stage >= 5:
                    for j in range(NLT):
                        t_ = hpool.tile([128, DH], F32R, tag="hn")
                        nc.scalar.dma_start(t_[:], Hn_d.ap()[j * 128:(j + 1) * 128, :])
                        hn.append(t_)
                    for e in range(NDT):
                        nc.gpsimd.dma_start(wkwT[:, e * DH:(e + 1) * DH],
                                            WkwT_d.ap()[e * 128:(e + 1) * 128, :])
            for cc in range(SPAN // 512):
                ps_ = lg.tile([T, 512], F32, tag="lg")
                for i in range(NDT):
                    nc.tensor.matmul(ps_[:], qkT[i][:], ht[(cc, i)][:],
                                     start=(i == 0), stop=(i == NDT - 1 and not with_mask))
                if with_mask:
                    nc.tensor.matmul(ps_[:], onesr[:], mbias[:, cc * 512:(cc + 1) * 512],
                                     start=False, stop=True)
                for u in range(2):
                    c = 2 * cc + u
                    nc.scalar.activation(expLs[c][:], ps_[:, u * 256:(u + 1) * 256],
                                         ActFn.Exp, accum_out=csums[c][:])
            if stage == 2:
                nc.sync.dma_start(zw_d.ap()[:T, :256], expLs[0][:])

        # ---- transpose expL into [l, t] f32r tiles; denominators ----
        if stage >= 3:
            expLT = []
            for j in range(NLT):
                ps_ = pj.tile([128, 512], F32, tag="pj")
                nc.tensor.transpose(ps_[:, :T], expLs[j // 2][:, (j % 2) * 128:(j % 2) * 128 + 128],
                                    ident[:T, :T])
                t_ = sb.tile([128, T], F32R, tag=f"eT{j}")
                nc.vector.tensor_copy(t_[:], ps_[:, :T])
                expLT.append(t_)
            recs = []
            for j in range(WLOC):
                d0_ = sexp.tile([T, 1], F32, tag=f"d0_{j}")
                d1_ = sexp.tile([T, 1], F32, tag=f"d1_{j}")
                rc_ = sexp.tile([T, 1], F32, tag=f"rc_{j}")
                nc.vector.tensor_add(d0_[:], csums[j][:], csums[j + 1][:])
                nc.vector.tensor_add(d1_[:], csums[j + 2][:], csums[j + 3][:])
                nc.vector.tensor_add(d0_[:], d0_[:], d1_[:])
                nc.vector.reciprocal(rc_[:], d0_[:])
                recs.append(rc_)
            if stage == 3:
                dbg = sb.tile([128, 2 * T], F32, tag="dbg")
                nc.vector.tensor_copy(dbg[:, :T], expLT[0][:].bitcast(F32))
                nc.vector.tensor_copy(dbg[:, T:2 * T], expLT[1][:].bitcast(F32))
                nc.sync.dma_start(zw_d.ap()[:128, :2 * T], dbg[:])
                nc.sync.dma_start(wl_d.ap()[:, :1], recs[0][:])

        # ---- qw -> qw^T(bf16) -> qw2 ----
        if stage >= 4:
            qw = sb.tile([T, DH], F32, tag="qw")
            for h in range(2):
                ps_ = zp.tile([T, 512], F32, tag="zp")
                for g in range(2):
                    nc.tensor.matmul(ps_[:], gt[:, g * T:(g + 1) * T],
                                     wqw[:, g * DH + h * 512:g * DH + (h + 1) * 512],
                                     start=(g == 0), stop=(g == 1))
                nc.scalar.activation(qw[:, h * 512:(h + 1) * 512], ps_[:], ActFn.Identity,
                                     scale=S_WIN)
            qwT = []
            for e in range(NDT):
                ps_ = pj.tile([128, 512], F32, tag="pj")
                nc.tensor.transpose(ps_[:, :T], qw[:, e * 128:(e + 1) * 128], ident[:T, :T])
                t_ = sb.tile([128, T], BF16, tag=f"qwT{e}")
                nc.vector.tensor_copy(t_[:], ps_[:, :T])
                qwT.append(t_)
            qw2 = sb.tile([T, DH], F32, tag="qw2")
            for h in range(2):
                ps_ = zp.tile([T, 512], F32, tag="zp")
                for e in range(NDT):
                    nc.tensor.matmul(ps_[:], qwT[e][:],
                                     wkwT[:, e * DH + h * 512:e * DH + (h + 1) * 512],
                                     start=(e == 0), stop=(e == NDT - 1))
                nc.scalar.activation(qw2[:, h * 512:(h + 1) * 512], ps_[:], ActFn.Identity)
            if stage == 4:
                nc.sync.dma_start(zw_d.ap()[:T, :DH], qw2[:])

        # ---- Zw per window (normalized in PSUM->SBUF copy), wlog inline ----
        if stage >= 5:
            wlog = sexp.tile([T, WLOC], F32, tag="wlog")
            scratch = sexp.tile([T, DH], F32, tag="scratch")
            for j in range(WLOC):
                t_ = sb.tile([T, DH], F32, tag="zw")
                ps_a = zp.tile([T, 512], F32, tag="zp")
                ps_b = zp.tile([T, 512], F32, tag="zp")
                pss = [ps_a, ps_b]
                for k in range(8):
                    for h in range(2):
                        nc.tensor.matmul(pss[h][:], expLT[2 * j + k][:],
                                         hn[2 * j + k][:, h * 512:(h + 1) * 512],
                                         start=(k == 0), stop=(k == 7))
                for h in range(2):
                    nc.vector.tensor_scalar_mul(t_[:, h * 512:(h + 1) * 512], pss[h][:],
                                                recs[j][:])
                nc.sync.dma_start(zw_d.ap()[j * T:(j + 1) * T, :], t_[:])
                if stage >= 7:
                    nc.vector.tensor_mul(scratch[:], t_[:], qw2[:])
                    nc.vector.reduce_sum(wlog[:, j:j + 1], scratch[:],
                                         axis=mybir.AxisListType.X)
            if stage >= 7:
                nc.gpsimd.dma_start(wl_d.ap(), wlog[:])

    nc.compile()
    return nc


def kernel(H, G, Wq_core, Wk_core, Wq_win, Wk_win, attn_mask):
    H = np.asarray(H, dtype=np.float32)
    G = np.asarray(G, dtype=np.float32)
    Wq_core = np.asarray(Wq_core, dtype=np.float32)
    Wk_core = np.asarray(Wk_core, dtype=np.float32)
    Wq_win = np.asarray(Wq_win, dtype=np.float32)
    Wk_win = np.asarray(Wk_win, dtype=np.float32)
    mask = np.asarray(attn_mask).astype(bool)

    with_mask = not bool(mask.all())
    key = ("k", with_mask)
    if key not in _CACHE:
        _CACHE[key] = _build(with_mask)
    nc = _CACHE[key]

    WkcT = np.ascontiguousarray(Wk_core.T)
    WkwT = np.ascontiguousarray(Wk_win.T).astype(ml_dtypes.bfloat16)

    in_maps = []
    for c in range(8):
        b, half = c // 2, c % 2
        lo = 0 if half == 0 else L - SPAN
        im = {
            "Hn": np.ascontiguousarray(H[b, lo:lo + SPAN, :]),
            "HT": np.ascontiguousarray(H[b].T[:, lo:lo + SPAN]),
            "GT": np.ascontiguousarray(G[b].T),
            "Wqc": Wq_core,
            "WkcT": WkcT,
            "Wqw": Wq_win,
            "WkwT": WkwT,
        }
        if with_mask:
            im["maskbias"] = np.where(mask[b, lo:lo + SPAN], 0.0, -1e9).astype(np.float32)[None, :]
            im["onesrow"] = np.ones((1, T), dtype=np.float32)
        in_maps.append(im)

    import os
    prof_dir = os.environ.get("BGW_PROFILE_DIR")
    if prof_dir:
        res = run_bass_kernel_spmd(nc, in_maps, core_ids=list(range(8)),
                                   trace=True, tmpdir=prof_dir)
    else:
        res = run_bass_kernel_spmd(nc, in_maps, core_ids=list(range(8)))
    kernel._last_result = res

    # ---- host combine: tiny cross-window softmax over W=13 ----
    Z = np.empty((B, T, DH), dtype=np.float32)
    for b in range(B):
        zw_full = np.empty((W, T, DH), dtype=np.float32)
        wl_full = np.empty((T, W), dtype=np.float32)
        for half in range(2):
            r = res.results[2 * b + half]
            zw = r["Zw_out"].reshape(WLOC, T, DH)
            wl = r["wlog_out"]
            w0 = 0 if half == 0 else W - WLOC
            zw_full[w0:w0 + WLOC] = zw
            wl_full[:, w0:w0 + WLOC] = wl
        m = wl_full.max(axis=1, keepdims=True)
        e = np.exp(wl_full - m)
        wsm = e / e.sum(axis=1, keepdims=True)          # [T, W]
        Z[b] = np.einsum("tw,wtd->td", wsm, zw_full)
    return Z



# revision 3
# speedup vs baseline: 1.5125x; 1.5125x over previous
"""Trainium2 Bass kernel v4 for nn_BucketedGoWatti.

Precision plan (measured ~7e-3 max-rel on the final output, gate 2e-2):
  - logits: qk split into fp8(e4m3) hi+lo residual pair (bf16-grade accuracy),
    H^T in plain fp8 e4m3; both matmuls DoubleRow (0.5 cyc/row).
  - alpha (exp of logits): bf16.
  - window sums: alpha-bf16 (lhsT) x Hn-e3m4 (rhs) mixed-dtype matmuls.
Device ships raw window sums Zwu (bf16) + per-256-chunk alpha sums (f32);
the tiny cross-window softmax combine (W=13, T=32) runs on the host.

Sharding: core c -> batch b=c//2, l-half c%2 (even: windows 0-6, l in
[0,2560); odd: windows 6-12, l in [1536,4096); dup window 6 dropped on host).
"""
import numpy as np
import ml_dtypes
from contextlib import ExitStack

import concourse.bacc as bacc
import concourse.tile as tile
import concourse.mybir as mybir
from concourse.bass_utils import run_bass_kernel_spmd

F32 = mybir.dt.float32
BF16 = mybir.dt.bfloat16
FP8 = mybir.dt.float8e4
E3M4 = mybir.dt.float8e3
ActFn = mybir.ActivationFunctionType
DR = mybir.MatmulPerfMode.DoubleRow

HN_E3 = True                         # Hn in e3m4 (else bf16 fallback)

B, L, T = 4, 4096, 32
DH, DG, DP = 1024, 256, 256
WIN, STRIDE = 1024, 256
W = (L - WIN) // STRIDE + 1          # 13
SPAN = 2560                          # per-core l-span
NCH = SPAN // 256                    # 10 chunks (256-wide)
NCC = SPAN // 512                    # 5 logits chunks (512-wide)
WLOC = 7                             # windows per core
QK_SCALE = 32.0                      # host prescale so fp8 qk stays normal
S_CORE = 1.0 / float(np.sqrt(DP))
S_WIN = 1.0 / float(np.sqrt(DH))

_CACHE = {}


def _bank_groups(c, bank):
    """Groups (0-3) of psum bank `bank` that 256-chunk c contributes to.

    Window w covers chunks w..w+3. Bank 0 = windows 0-3, bank 1 = 4-6."""
    lo, hi = max(c - 3, 4 * bank), min(c, 4 * bank + 3 if bank == 0 else 6)
    if lo > hi:
        return None
    return (lo - 4 * bank, hi - 4 * bank)


_PATTERNS = sorted({_bank_groups(c, b) for c in range(NCH) for b in range(2)}
                   - {None})
_PAT_IDX = {p: i for i, p in enumerate(_PATTERNS)}
_LAST_CB = {bank: max(c for c in range(NCH) if _bank_groups(c, bank))
            for bank in range(2)}


def host_pat():
    npat = len(_PATTERNS)
    out = np.zeros((T, npat * 128), dtype=np.float32)
    eye = np.eye(T, dtype=np.float32)
    for (lo, hi), i in _PAT_IDX.items():
        for g in range(lo, hi + 1):
            out[:, i * 128 + g * T:i * 128 + (g + 1) * T] = eye
    return out.astype(ml_dtypes.bfloat16)


def _build(with_mask: bool):
    nc = bacc.Bacc("TRN2", debug=False, target_bir_lowering=False)
    npat = len(_PATTERNS)
    hn_dt = E3M4 if HN_E3 else BF16
    hn_bpe = 1 if HN_E3 else 2

    HT_d = nc.dram_tensor("HT", [20 * 128, 1024], FP8, kind="ExternalInput")
    Hn_d = nc.dram_tensor("Hn", [10 * 128, 2048], hn_dt, kind="ExternalInput")
    qkT_d = nc.dram_tensor("qkT", [128, 2 * 8 * T], FP8, kind="ExternalInput")
    pat_d = nc.dram_tensor("pat", [T, npat * 128], BF16, kind="ExternalInput")
    if with_mask:
        mb_d = nc.dram_tensor("maskbias", [1, SPAN], BF16, kind="ExternalInput")
        ones_d = nc.dram_tensor("onesrow", [1, T], BF16, kind="ExternalInput")
    zw_d = nc.dram_tensor("Zwu_out", [2 * 128, DH], BF16, kind="ExternalOutput")
    cs_d = nc.dram_tensor("csum_out", [T, NCH], F32, kind="ExternalOutput")

    hq = [nc.sync, nc.gpsimd]       # H-tile DMA queues

    if True:
        tc = tile.TileContext(nc)
        tc.__enter__()
        ctx = ExitStack()
        const = ctx.enter_context(tc.tile_pool(name="const", bufs=1))
        htp = ctx.enter_context(tc.tile_pool(name="htp", bufs=1))
        hnp = ctx.enter_context(tc.tile_pool(name="hnp", bufs=1))
        sb = ctx.enter_context(tc.tile_pool(name="sb", bufs=1))
        lg = ctx.enter_context(tc.tile_pool(name="lg", bufs=2, space="PSUM"))
        tp = ctx.enter_context(tc.tile_pool(name="tp", bufs=2, space="PSUM"))
        zw = ctx.enter_context(tc.tile_pool(name="zw", bufs=1, space="PSUM"))

        qkT = const.tile([128, 2 * 8 * T], FP8, tag="qkT")
        pat = const.tile([T, npat * 128], BF16, tag="pat")
        nc.sync.dma_start(qkT[:], qkT_d.ap())
        nc.scalar.dma_start(pat[:], pat_d.ap())
        if with_mask:
            mbias = const.tile([1, SPAN], BF16, tag="mbias")
            onesr = const.tile([1, T], BF16, tag="onesr")
            nc.scalar.dma_start(mbias[:], mb_d.ap())
            nc.scalar.dma_start(onesr[:], ones_d.ap())

        warm = const.tile([1, 1], F32, tag="warm")
        nc.vector.memset(warm[:], 0.0)
        nc.scalar.activation(warm[:], warm[:], ActFn.Exp)

        expL = [sb.tile([T, 256], BF16, tag=f"expL{c}", name=f"expL{c}")
                for c in range(NCH)]
        csum = sb.tile([T, NCH], F32, tag="csum")
        elt = {}                          # (c, bank) -> [128, 256] bf16 lhsT
        zwu = {(b, h): zw.tile([128, 512], F32, tag=f"zwu{b}{h}",
                               name=f"zwu{b}{h}")
               for b in range(2) for h in range(2)}
        started = {b: False for b in range(2)}

        zb = {(b, h): sb.tile([128, 512], BF16, tag=f"zb{b}{h}",
                              name=f"zb{b}{h}")
              for b in range(2) for h in range(2)}

        ht2 = {}    # (m, cc) -> [128, 1024] fp8, halves = d-tiles 2m, 2m+1
        hn2 = {}    # c -> [128, 2048(bytes/bpe)] e3m4/bf16, l-tiles 2c, 2c+1

        def dma_chunk(cc):
            for m in range(4):
                t_ = htp.tile([128, 1024], FP8, tag=f"ht{m}_{cc}",
                              name=f"ht{m}_{cc}")
                k = cc * 4 + m
                hq[m % 2].dma_start(t_[:], HT_d.ap()[k * 128:(k + 1) * 128, :])
                ht2[(m, cc)] = t_
            for c in (2 * cc, 2 * cc + 1):
                t_ = hnp.tile([128, 2048], hn_dt, tag=f"hn{c}", name=f"hn{c}")
                hq[c % 2].dma_start(t_[:],
                                    Hn_d.ap()[c * 128:(c + 1) * 128, :])
                hn2[c] = t_

        pend = []

        def emit_c4(c, bank):
            first, last = not started[bank], c == _LAST_CB[bank]
            for i in range(2):
                lhs = elt[(c, bank)][:, i * 128:(i + 1) * 128]
                for h in range(2):
                    nc.tensor.matmul(
                        zwu[(bank, h)][:], lhs,
                        hn2[c][:, i * 1024 + h * 512:i * 1024 + (h + 1) * 512],
                        start=first and i == 0, stop=last and i == 1)
            started[bank] = True

        def bank_out(bank):
            """Ship one bank's window sums: psum -> bf16 sbuf -> DRAM."""
            nc.vector.tensor_copy(zb[(bank, 0)][:], zwu[(bank, 0)][:])
            nc.scalar.activation(zb[(bank, 1)][:], zwu[(bank, 1)][:],
                                 ActFn.Identity)
            nc.sync.dma_start(zw_d.ap()[bank * 128:(bank + 1) * 128, :512],
                              zb[(bank, 0)][:])
            nc.gpsimd.dma_start(zw_d.ap()[bank * 128:(bank + 1) * 128, 512:],
                                zb[(bank, 1)][:])

        def transposes_and_c4(c):
            for bank in range(2):
                rng = _bank_groups(c, bank)
                if rng is None:
                    continue
                pid = _PAT_IDX[rng]
                t_ = sb.tile([128, 256], BF16, tag=f"elt{c}_{bank}",
                             name=f"elt{c}_{bank}")
                ps_ = tp.tile([128, 256], F32, tag="tp")
                for i in range(2):
                    nc.tensor.matmul(
                        ps_[:, i * 128:(i + 1) * 128],
                        expL[c][:, i * 128:(i + 1) * 128],
                        pat[:, pid * 128:(pid + 1) * 128],
                        start=True, stop=True)
                nc.vector.tensor_copy(t_[:], ps_[:])
                if pend:
                    emit_c4(*pend.pop(0))
                elt[(c, bank)] = t_
                pend.append((c, bank))

        # ---- main pipeline ----
        for cc in range(NCC):
            dma_chunk(cc)
            ps_ = lg.tile([T, 512], F32, tag="lg")
            for r in range(2):           # qk hi then lo residual
                for m in range(4):
                    nc.tensor.matmul(
                        ps_[:],
                        qkT[:, r * 256 + m * 64:r * 256 + (m + 1) * 64]
                        .rearrange("p (i t) -> p i t", i=2),
                        ht2[(m, cc)][:].rearrange("p (i n) -> p i n", i=2),
                        start=(r == 0 and m == 0),
                        stop=(r == 1 and m == 3 and not with_mask),
                        perf_mode=DR)
            if with_mask:
                nc.tensor.matmul(ps_[:], onesr[:],
                                 mbias[:, cc * 512:(cc + 1) * 512],
                                 start=False, stop=True)
            for u in range(2):
                c = 2 * cc + u
                nc.scalar.activation(expL[c][:],
                                     ps_[:, u * 256:(u + 1) * 256], ActFn.Exp,
                                     scale=1.0 / QK_SCALE,
                                     accum_out=csum[:, c:c + 1])
            if cc >= 1:
                transposes_and_c4(2 * (cc - 1))
                transposes_and_c4(2 * (cc - 1) + 1)
            if cc == NCC - 1:
                bank_out(0)      # bank0 closed after chunk 6 (cc=3)
        nc.scalar.dma_start(cs_d.ap(), csum[:])
        transposes_and_c4(2 * (NCC - 1))
        transposes_and_c4(2 * (NCC - 1) + 1)
        while pend:
            emit_c4(*pend.pop(0))
        bank_out(1)

        ctx.close()
        tc.__exit__(None, None, None)
    nc.compile()
    return nc


def _host_pack(H, G, Wq_core, Wk_core, mask, with_mask):
    qk32 = {b: (G[b] @ Wq_core) @ Wk_core.T * (S_CORE * QK_SCALE)
            for b in range(B)}
    pat = host_pat()
    fp8 = ml_dtypes.float8_e4m3
    hn_np = ml_dtypes.float8_e3m4 if HN_E3 else ml_dtypes.bfloat16
    in_maps = []
    for c in range(8):
        b, half = c // 2, c % 2
        lo = 0 if half == 0 else L - SPAN

        def packq(x):   # [T, DH] -> [128, 8*T] d-tile-major
            return np.ascontiguousarray(
                x.T.reshape(8, 128, T).transpose(1, 0, 2).reshape(128, 8 * T))

        hi = qk32[b].astype(fp8)
        lop = (qk32[b] - hi.astype(np.float32)).astype(fp8)
        qkT = np.concatenate([packq(hi.astype(np.float32)),
                              packq(lop.astype(np.float32))], axis=1)
        im = {
            "HT": np.ascontiguousarray(
                H[b].T[:, lo:lo + SPAN].reshape(4, 2, 128, 5, 512)
                .transpose(3, 0, 2, 1, 4).reshape(20 * 128, 1024)).astype(fp8),
            "Hn": np.ascontiguousarray(
                H[b, lo:lo + SPAN, :].reshape(10, 2, 128, 1024)
                .transpose(0, 2, 1, 3).reshape(10 * 128, 2048)).astype(hn_np),
            "qkT": qkT.astype(fp8),
            "pat": pat,
        }
        if with_mask:
            im["maskbias"] = np.where(mask[b, lo:lo + SPAN], 0.0, -30000.0
                                      ).astype(ml_dtypes.bfloat16)[None, :]
            im["onesrow"] = np.ones((1, T), dtype=ml_dtypes.bfloat16)
        in_maps.append(im)
    return in_maps


def _host_combine(results, G, Wq_win, Wk_win):
    qw2 = {b: ((G[b] @ Wq_win) @ Wk_win.T * S_WIN).astype(np.float64)
           for b in range(B)}
    Z = np.empty((B, T, DH), dtype=np.float32)
    for b in range(B):
        Zw = np.empty((W, T, DH), dtype=np.float64)
        for half in range(2):
            r = results[2 * b + half]
            zwu = r["Zwu_out"].astype(np.float64)      # [2*128, DH]
            cs = r["csum_out"].astype(np.float64)      # [T, NCH]
            for w in range(WLOC):
                if half == 1 and w == 0:
                    continue                           # dup of global w6
                bank, g = w // 4, w % 4
                gw = w if half == 0 else w + 6
                den = cs[:, w:w + 4].sum(axis=1)
                block = zwu[bank * 128 + g * T: bank * 128 + (g + 1) * T]
                Zw[gw] = block / den[:, None]
        wlog = np.einsum("wtd,td->tw", Zw, qw2[b])     # [T, W]
        wlog -= wlog.max(axis=1, keepdims=True)
        e = np.exp(wlog)
        wsm = e / e.sum(axis=1, keepdims=True)
        Z[b] = np.einsum("tw,wtd->td", wsm, Zw).astype(np.float32)
    return Z


def kernel(H, G, Wq_core, Wk_core, Wq_win, Wk_win, attn_mask):
    H = np.asarray(H, dtype=np.float32)
    G = np.asarray(G, dtype=np.float32)
    Wq_core = np.asarray(Wq_core, dtype=np.float32)
    Wk_core = np.asarray(Wk_core, dtype=np.float32)
    Wq_win = np.asarray(Wq_win, dtype=np.float32)
    Wk_win = np.asarray(Wk_win, dtype=np.float32)
    mask = np.asarray(attn_mask).astype(bool)

    with_mask = not bool(mask.all())
    key = ("k", with_mask)
    if key not in _CACHE:
        _CACHE[key] = _build(with_mask)
    nc = _CACHE[key]

    in_maps = _host_pack(H, G, Wq_core, Wk_core, mask, with_mask)
    import os
    prof_dir = os.environ.get("BGW_PROFILE_DIR")
    if prof_dir:
        res = run_bass_kernel_spmd(nc, in_maps, core_ids=list(range(8)),
                                   trace=True, tmpdir=prof_dir)
    else:
        res = run_bass_kernel_spmd(nc, in_maps, core_ids=list(range(8)))
    kernel._last_result = res
    return _host_combine(res.results, G, Wq_win, Wk_win)


# revision 4
# speedup vs baseline: 1.5395x; 1.0179x over previous
"""Trainium2 Bass kernel v4 for nn_BucketedGoWatti.

Precision plan (measured ~7e-3 max-rel on the final output, gate 2e-2):
  - logits: qk split into fp8(e4m3) hi+lo residual pair (bf16-grade accuracy),
    H^T in plain fp8 e4m3; both matmuls DoubleRow (0.5 cyc/row).
  - alpha (exp of logits): bf16.
  - window sums: alpha-bf16 (lhsT) x Hn-e3m4 (rhs) mixed-dtype matmuls.
Device ships raw window sums Zwu (bf16) + per-256-chunk alpha sums (f32);
the tiny cross-window softmax combine (W=13, T=32) runs on the host.

Sharding: core c -> batch b=c//2, l-half c%2 (even: windows 0-6, l in
[0,2560); odd: windows 6-12, l in [1536,4096); dup window 6 dropped on host).
"""
import numpy as np
import ml_dtypes
from contextlib import ExitStack

import concourse.bacc as bacc
import concourse.tile as tile
import concourse.mybir as mybir
from concourse.bass_utils import run_bass_kernel_spmd

F32 = mybir.dt.float32
BF16 = mybir.dt.bfloat16
FP8 = mybir.dt.float8e4
E3M4 = mybir.dt.float8e3
ActFn = mybir.ActivationFunctionType
DR = mybir.MatmulPerfMode.DoubleRow

HN_E3 = True                         # Hn in e3m4 (else bf16 fallback)

B, L, T = 4, 4096, 32
DH, DG, DP = 1024, 256, 256
WIN, STRIDE = 1024, 256
W = (L - WIN) // STRIDE + 1          # 13
SPAN = 2560                          # per-core l-span
NCH = SPAN // 256                    # 10 chunks (256-wide)
NCC = SPAN // 512                    # 5 logits chunks (512-wide)
WLOC = 7                             # windows per core
QK_SCALE = 32.0                      # host prescale so fp8 qk stays normal
S_CORE = 1.0 / float(np.sqrt(DP))
S_WIN = 1.0 / float(np.sqrt(DH))

_CACHE = {}


def _bank_groups(c, bank):
    """Groups (0-3) of psum bank `bank` that 256-chunk c contributes to.

    Window w covers chunks w..w+3. Bank 0 = windows 0-3, bank 1 = 4-6."""
    lo, hi = max(c - 3, 4 * bank), min(c, 4 * bank + 3 if bank == 0 else 6)
    if lo > hi:
        return None
    return (lo - 4 * bank, hi - 4 * bank)


_PATTERNS = sorted({_bank_groups(c, b) for c in range(NCH) for b in range(2)}
                   - {None})
_PAT_IDX = {p: i for i, p in enumerate(_PATTERNS)}
_LAST_CB = {bank: max(c for c in range(NCH) if _bank_groups(c, bank))
            for bank in range(2)}


def host_pat():
    npat = len(_PATTERNS)
    out = np.zeros((T, npat * 128), dtype=np.float32)
    eye = np.eye(T, dtype=np.float32)
    for (lo, hi), i in _PAT_IDX.items():
        for g in range(lo, hi + 1):
            out[:, i * 128 + g * T:i * 128 + (g + 1) * T] = eye
    return out.astype(ml_dtypes.bfloat16)


def _build(with_mask: bool):
    nc = bacc.Bacc("TRN2", debug=False, target_bir_lowering=False)
    npat = len(_PATTERNS)
    hn_dt = E3M4 if HN_E3 else BF16
    hn_bpe = 1 if HN_E3 else 2

    HT_d = nc.dram_tensor("HT", [10 * 128, 2048], FP8, kind="ExternalInput")
    Hn_d = nc.dram_tensor("Hn", [10 * 128, 2048], hn_dt, kind="ExternalInput")
    qkT_d = nc.dram_tensor("qkT", [128, 2 * 8 * T], FP8, kind="ExternalInput")
    pat_d = nc.dram_tensor("pat", [T, npat * 128], BF16, kind="ExternalInput")
    if with_mask:
        mb_d = nc.dram_tensor("maskbias", [1, SPAN], BF16, kind="ExternalInput")
        ones_d = nc.dram_tensor("onesrow", [1, T], BF16, kind="ExternalInput")
    zw_d = nc.dram_tensor("Zwu_out", [2 * 128, DH], BF16, kind="ExternalOutput")
    cs_d = nc.dram_tensor("csum_out", [T, NCH], F32, kind="ExternalOutput")

    hq = [nc.sync, nc.gpsimd]       # H-tile DMA queues

    if True:
        tc = tile.TileContext(nc)
        tc.__enter__()
        ctx = ExitStack()
        const = ctx.enter_context(tc.tile_pool(name="const", bufs=1))
        htp = ctx.enter_context(tc.tile_pool(name="htp", bufs=1))
        hnp = ctx.enter_context(tc.tile_pool(name="hnp", bufs=1))
        sb = ctx.enter_context(tc.tile_pool(name="sb", bufs=1))
        lg = ctx.enter_context(tc.tile_pool(name="lg", bufs=2, space="PSUM"))
        tp = ctx.enter_context(tc.tile_pool(name="tp", bufs=2, space="PSUM"))
        zw = ctx.enter_context(tc.tile_pool(name="zw", bufs=1, space="PSUM"))

        qkT = const.tile([128, 2 * 8 * T], FP8, tag="qkT")
        pat = const.tile([T, npat * 128], BF16, tag="pat")
        nc.sync.dma_start(qkT[:], qkT_d.ap())
        nc.scalar.dma_start(pat[:], pat_d.ap())
        if with_mask:
            mbias = const.tile([1, SPAN], BF16, tag="mbias")
            onesr = const.tile([1, T], BF16, tag="onesr")
            nc.scalar.dma_start(mbias[:], mb_d.ap())
            nc.scalar.dma_start(onesr[:], ones_d.ap())

        warm = const.tile([1, 1], F32, tag="warm")
        nc.vector.memset(warm[:], 0.0)
        nc.scalar.activation(warm[:], warm[:], ActFn.Exp)
        wa = const.tile([128, 32], BF16, tag="wa")
        wb = const.tile([128, 512], BF16, tag="wb")
        nc.vector.memset(wa[:], 0.0)
        nc.vector.memset(wb[:], 0.0)
        wps = tp.tile([T, 512], F32, tag="tp", name="warm_ps")
        for _ in range(2):
            nc.tensor.matmul(wps[:], wa[:], wb[:], start=True, stop=True)

        expL = [sb.tile([T, 256], BF16, tag=f"expL{c}", name=f"expL{c}")
                for c in range(NCH)]
        csum = sb.tile([T, NCH], F32, tag="csum")
        elt = {}                          # (c, bank) -> [128, 256] bf16 lhsT
        zwu = {(b, h): zw.tile([128, 512], F32, tag=f"zwu{b}{h}",
                               name=f"zwu{b}{h}")
               for b in range(2) for h in range(2)}
        started = {b: False for b in range(2)}

        zb = {(b, h): sb.tile([128, 512], BF16, tag=f"zb{b}{h}",
                              name=f"zb{b}{h}")
              for b in range(2) for h in range(2)}

        ht2 = {}    # (m, cc) -> [128, 1024] fp8, halves = d-tiles 2m, 2m+1
        hn2 = {}    # c -> [128, 2048(bytes/bpe)] e3m4/bf16, l-tiles 2c, 2c+1

        def dma_chunk(cc):
            for qq in range(2):
                t_ = htp.tile([128, 2048], FP8, tag=f"ht{qq}_{cc}",
                              name=f"ht{qq}_{cc}")
                k = cc * 2 + qq
                hq[qq].dma_start(t_[:], HT_d.ap()[k * 128:(k + 1) * 128, :])
                ht2[(qq, cc)] = t_
            for c in (2 * cc, 2 * cc + 1):
                t_ = hnp.tile([128, 2048], hn_dt, tag=f"hn{c}", name=f"hn{c}")
                hq[c % 2].dma_start(t_[:],
                                    Hn_d.ap()[c * 128:(c + 1) * 128, :])
                hn2[c] = t_

        pend = []

        def emit_c4(c, bank):
            first, last = not started[bank], c == _LAST_CB[bank]
            for h in range(2):
                for i in range(2):
                    nc.tensor.matmul(
                        zwu[(bank, h)][:],
                        elt[(c, bank)][:, i * 128:(i + 1) * 128],
                        hn2[c][:, i * 1024 + h * 512:i * 1024 + (h + 1) * 512],
                        start=first and i == 0, stop=last and i == 1)
            started[bank] = True

        def bank_out(bank):
            """Ship one bank's window sums: psum -> bf16 sbuf -> DRAM."""
            nc.vector.tensor_copy(zb[(bank, 0)][:], zwu[(bank, 0)][:])
            nc.scalar.activation(zb[(bank, 1)][:], zwu[(bank, 1)][:],
                                 ActFn.Identity)
            nc.sync.dma_start(zw_d.ap()[bank * 128:(bank + 1) * 128, :512],
                              zb[(bank, 0)][:])
            nc.gpsimd.dma_start(zw_d.ap()[bank * 128:(bank + 1) * 128, 512:],
                                zb[(bank, 1)][:])

        def transposes_and_c4(c):
            for bank in range(2):
                rng = _bank_groups(c, bank)
                if rng is None:
                    continue
                pid = _PAT_IDX[rng]
                t_ = sb.tile([128, 256], BF16, tag=f"elt{c}_{bank}",
                             name=f"elt{c}_{bank}")
                ps_ = tp.tile([128, 256], F32, tag="tp")
                for i in range(2):
                    nc.tensor.matmul(
                        ps_[:, i * 128:(i + 1) * 128],
                        expL[c][:, i * 128:(i + 1) * 128],
                        pat[:, pid * 128:(pid + 1) * 128],
                        start=True, stop=True)
                nc.vector.tensor_copy(t_[:], ps_[:])
                if pend:
                    emit_c4(*pend.pop(0))
                elt[(c, bank)] = t_
                pend.append((c, bank))

        # ---- main pipeline ----
        for cc in range(NCC):
            dma_chunk(cc)
            ps_ = lg.tile([T, 512], F32, tag="lg")
            for r in range(2):           # qk hi then lo residual
                for m in range(4):
                    nc.tensor.matmul(
                        ps_[:],
                        qkT[:, r * 256 + m * 64:r * 256 + (m + 1) * 64]
                        .rearrange("p (i t) -> p i t", i=2),
                        ht2[(m // 2, cc)][:, (m % 2) * 1024:(m % 2 + 1) * 1024]
                        .rearrange("p (i n) -> p i n", i=2),
                        start=(r == 0 and m == 0),
                        stop=(r == 1 and m == 3 and not with_mask),
                        perf_mode=DR)
            if with_mask:
                nc.tensor.matmul(ps_[:], onesr[:],
                                 mbias[:, cc * 512:(cc + 1) * 512],
                                 start=False, stop=True)
            for u in range(2):
                c = 2 * cc + u
                nc.scalar.activation(expL[c][:],
                                     ps_[:, u * 256:(u + 1) * 256], ActFn.Exp,
                                     scale=1.0 / QK_SCALE,
                                     accum_out=csum[:, c:c + 1])
            if cc >= 1:
                transposes_and_c4(2 * (cc - 1))
                transposes_and_c4(2 * (cc - 1) + 1)
            if cc == NCC - 1:
                bank_out(0)      # bank0 closed after chunk 6 (cc=3)
        nc.scalar.dma_start(cs_d.ap(), csum[:])
        transposes_and_c4(2 * (NCC - 1))
        transposes_and_c4(2 * (NCC - 1) + 1)
        while pend:
            emit_c4(*pend.pop(0))
        bank_out(1)

        ctx.close()
        tc.__exit__(None, None, None)
    nc.compile()
    return nc


def _host_pack(H, G, Wq_core, Wk_core, mask, with_mask):
    qk32 = {b: (G[b] @ Wq_core) @ Wk_core.T * (S_CORE * QK_SCALE)
            for b in range(B)}
    pat = host_pat()
    fp8 = ml_dtypes.float8_e4m3
    hn_np = ml_dtypes.float8_e3m4 if HN_E3 else ml_dtypes.bfloat16
    in_maps = []
    for c in range(8):
        b, half = c // 2, c % 2
        lo = 0 if half == 0 else L - SPAN

        def packq(x):   # [T, DH] -> [128, 8*T] d-tile-major
            return np.ascontiguousarray(
                x.T.reshape(8, 128, T).transpose(1, 0, 2).reshape(128, 8 * T))

        hi = qk32[b].astype(fp8)
        lop = (qk32[b] - hi.astype(np.float32)).astype(fp8)
        qkT = np.concatenate([packq(hi.astype(np.float32)),
                              packq(lop.astype(np.float32))], axis=1)
        im = {
            "HT": np.ascontiguousarray(
                H[b].T[:, lo:lo + SPAN].reshape(2, 2, 2, 128, 5, 512)
                .transpose(4, 0, 3, 1, 2, 5).reshape(10 * 128, 2048)).astype(fp8),
            "Hn": np.ascontiguousarray(
                H[b, lo:lo + SPAN, :].reshape(10, 2, 128, 1024)
                .transpose(0, 2, 1, 3).reshape(10 * 128, 2048)).astype(hn_np),
            "qkT": qkT.astype(fp8),
            "pat": pat,
        }
        if with_mask:
            im["maskbias"] = np.where(mask[b, lo:lo + SPAN], 0.0, -30000.0
                                      ).astype(ml_dtypes.bfloat16)[None, :]
            im["onesrow"] = np.ones((1, T), dtype=ml_dtypes.bfloat16)
        in_maps.append(im)
    return in_maps


def _host_combine(results, G, Wq_win, Wk_win):
    qw2 = {b: ((G[b] @ Wq_win) @ Wk_win.T * S_WIN).astype(np.float64)
           for b in range(B)}
    Z = np.empty((B, T, DH), dtype=np.float32)
    for b in range(B):
        Zw = np.empty((W, T, DH), dtype=np.float64)
        for half in range(2):
            r = results[2 * b + half]
            zwu = r["Zwu_out"].astype(np.float64)      # [2*128, DH]
            cs = r["csum_out"].astype(np.float64)      # [T, NCH]
            for w in range(WLOC):
                if half == 1 and w == 0:
                    continue                           # dup of global w6
                bank, g = w // 4, w % 4
                gw = w if half == 0 else w + 6
                den = cs[:, w:w + 4].sum(axis=1)
                block = zwu[bank * 128 + g * T: bank * 128 + (g + 1) * T]
                Zw[gw] = block / den[:, None]
        wlog = np.einsum("wtd,td->tw", Zw, qw2[b])     # [T, W]
        wlog -= wlog.max(axis=1, keepdims=True)
        e = np.exp(wlog)
        wsm = e / e.sum(axis=1, keepdims=True)
        Z[b] = np.einsum("tw,wtd->td", wsm, Zw).astype(np.float32)
    return Z


def kernel(H, G, Wq_core, Wk_core, Wq_win, Wk_win, attn_mask):
    H = np.asarray(H, dtype=np.float32)
    G = np.asarray(G, dtype=np.float32)
    Wq_core = np.asarray(Wq_core, dtype=np.float32)
    Wk_core = np.asarray(Wk_core, dtype=np.float32)
    Wq_win = np.asarray(Wq_win, dtype=np.float32)
    Wk_win = np.asarray(Wk_win, dtype=np.float32)
    mask = np.asarray(attn_mask).astype(bool)

    with_mask = not bool(mask.all())
    key = ("k", with_mask)
    if key not in _CACHE:
        _CACHE[key] = _build(with_mask)
    nc = _CACHE[key]

    in_maps = _host_pack(H, G, Wq_core, Wk_core, mask, with_mask)
    import os
    prof_dir = os.environ.get("BGW_PROFILE_DIR")
    if prof_dir:
        res = run_bass_kernel_spmd(nc, in_maps, core_ids=list(range(8)),
                                   trace=True, tmpdir=prof_dir)
    else:
        res = run_bass_kernel_spmd(nc, in_maps, core_ids=list(range(8)))
    kernel._last_result = res
    return _host_combine(res.results, G, Wq_win, Wk_win)
